# revision 1
# baseline (speedup 1.0000x reference)
"""Trainium2 Bass kernel for nn_Attention_33054068310137 (8-core SPMD).

Math: the reference computes, per (head h, batch b):
    blend = (1-w_h) * cosine + w_h * cov        # [N, N]
    out_h = blend @ fv                          # [N, DH]
with cosine[n,m] = (fq[n].fk[m])/(|fq[n]||fk[m]|) and
cov[n,m] = ((fq[n]-qm[n]).(fk[m]-km[m]))/DH.

Both score matrices are rank-DH outer products of per-row-rescaled
projections, so with
    A_q[n] = [fq[n]/|fq[n]| ; fq[n]-qm[n]]      # [N, 2*DH]
    A_k[m] = [fk[m]/|fk[m]| ; fk[m]-km[m]]
    S      = A_k^T @ fv                         # [2*DH, DH]  (tiny!)
    S'     = diag([(1-w)*1_64 ; (w/DH)*1_64]) @ S
we get out_h = A_q @ S' without ever materializing the N x N matrices.

Sharding: rows (B*N = 4096) split across 8 cores (cores 0-3 hold batch 0,
cores 4-7 batch 1). Two collectives:
  AR2: 2-group (per-batch) AllReduce of the S partials (bf16, 128KB)
  AR1: all-8 AllReduce of the global head-mean partials qg/kg (f32, 4KB)
Everything else is core-local. Weights are replicated.

Device layouts: LayerNorm + all per-row scalings run row-major
([row, feat] with rows on partitions); matmuls get their contraction
axis onto partitions via PE transposes. The final output is produced
transposed ([D, R] per core) and un-transposed on the host.
"""

import os
import numpy as np

H, DH, B, N, D = 8, 64, 2, 2048, 512
EPS = 1e-5
N_CORES = 8
R = (B * N) // N_CORES  # rows per core = 512
P = 128                 # SBUF partitions

COMPUTE = os.environ.get("BASSK_COMPUTE", "bf16")  # "bf16" | "f32"

_CACHE = {}


def _build_program():
    import concourse.bacc as bacc
    import concourse.bass as bass
    import concourse.mybir as mybir
    import concourse.tile as tile
    from concourse.masks import make_identity
    from contextlib import ExitStack

    f32 = mybir.dt.float32
    bf16 = mybir.dt.bfloat16
    CDT = bf16 if COMPUTE == "bf16" else f32
    AX = mybir.AxisListType
    OP = mybir.AluOpType
    AF = mybir.ActivationFunctionType

    nc = bacc.Bacc("TRN2", target_bir_lowering=False, debug=False,
                   enable_asserts=True, num_devices=N_CORES)

    # ---- external I/O (per-core shapes) ----
    q_ext = nc.dram_tensor("q", [R, D], f32, kind="ExternalInput")
    k_ext = nc.dram_tensor("k", [R, D], f32, kind="ExternalInput")
    v_ext = nc.dram_tensor("v", [R, D], f32, kind="ExternalInput")
    winT_ext = nc.dram_tensor("W_inT", [D, D], f32, kind="ExternalInput")    # [d, j]
    woutT_ext = nc.dram_tensor("W_outT", [D, D], f32, kind="ExternalInput")  # [j, dcol]
    lng_ext = nc.dram_tensor("ln_g", [D], f32, kind="ExternalInput")
    lnb_ext = nc.dram_tensor("ln_b", [D], f32, kind="ExternalInput")
    bout_ext = nc.dram_tensor("b_out", [D], f32, kind="ExternalInput")
    w1T_ext = nc.dram_tensor("wp_w1T", [2 * DH, DH], f32, kind="ExternalInput")
    b1_ext = nc.dram_tensor("wp_b1", [DH], f32, kind="ExternalInput")
    wlg_ext = nc.dram_tensor("wp_ln_g", [DH], f32, kind="ExternalInput")
    wlb_ext = nc.dram_tensor("wp_ln_b", [DH], f32, kind="ExternalInput")
    w2_ext = nc.dram_tensor("wp_w2", [DH], f32, kind="ExternalInput")
    b2_ext = nc.dram_tensor("wp_b2", [1], f32, kind="ExternalInput")
    bmask_ext = nc.dram_tensor("bmask", [2], f32, kind="ExternalInput")
    out_ext = nc.dram_tensor("out", [D, R], f32, kind="ExternalOutput")  # transposed

    NT = R // P  # row tiles per core = 4
    ND = D // P  # feature tiles = 4

    def _patch_pool(pool):
        orig = pool.tile
        def tile_(shape, dtype, tag, **kw):
            return orig(shape, dtype, name=tag, tag=tag, **kw)
        pool.tile = tile_
        return pool

    with tile.TileContext(nc) as tc, ExitStack() as ctx:
        consts = _patch_pool(ctx.enter_context(tc.tile_pool(name="consts", bufs=1)))
        wp = _patch_pool(ctx.enter_context(tc.tile_pool(name="wp", bufs=1)))
        work = _patch_pool(ctx.enter_context(tc.tile_pool(name="work", bufs=4)))
        keep = _patch_pool(ctx.enter_context(tc.tile_pool(name="keep", bufs=1)))
        psum = _patch_pool(ctx.enter_context(tc.tile_pool(name="psum", bufs=1, space="PSUM")))
        pss = _patch_pool(ctx.enter_context(tc.tile_pool(name="pss", bufs=1, space="PSUM")))
        dram = _patch_pool(ctx.enter_context(tc.tile_pool(name="dram", bufs=1, space="DRAM")))

        if True:
            # ---------------- constants ----------------
            ident = consts.tile([P, P], CDT, tag="ident")
            make_identity(nc, ident[:])
            ones_row = consts.tile([1, P], CDT, tag="ones_row")      # K=1 bias matmul lhsT
            nc.vector.memset(ones_row[:], 1.0)
            colsum_lhsT = consts.tile([P, 1], CDT, tag="colsum_lhsT")  # global-mean matmul
            nc.vector.memset(colsum_lhsT[:], 1.0 / (B * N))
            eps_t = consts.tile([P, 1], f32, tag="eps_t")
            nc.vector.memset(eps_t[:], EPS)
            ind_top = consts.tile([1, P], bf16, tag="ind_top")
            nc.vector.memset(ind_top[:], 0.0)
            nc.vector.memset(ind_top[:, 0:DH], 1.0)
            eighth1 = consts.tile([1, 1], CDT, tag="eighth1")
            nc.vector.memset(eighth1[:], R / float(B * N))
            ident8 = consts.tile([H, H], f32, tag="ident8")
            make_identity(nc, ident8[:])
            ind_bot = consts.tile([1, P], bf16, tag="ind_bot")
            nc.vector.memset(ind_bot[:], 0.0)
            nc.vector.memset(ind_bot[:, DH:P], 1.0)

            # prime the collective engine: tiny AllReduce absorbs the first-trigger
            # ncfw wakeup + launch-skew barrier while the PE/DMA pipeline fills
            cc_prime = consts.tile([1, 64], bf16, tag="cc_prime")
            nc.vector.memset(cc_prime[:], 0.0)
            prime_in = dram.tile([64], bf16, tag="prime_in")
            prime_out = dram.tile([64], bf16, tag="prime_out")
            nc.gpsimd.dma_start(prime_in[:].unsqueeze(0), cc_prime[:])
            nc.gpsimd.collective_compute(
                "AllReduce", OP.add,
                replica_groups=[[0, 1], [2, 3], [4, 5], [6, 7]],
                ins=[prime_in.opt()], outs=[prime_out.opt()])

            # PE clock warm-up (HAM): dense matmuls during initial DMA window
            warm_ps = psum.tile([P, P], f32, tag="trps", bufs=3)
            for wi in range(40):
                nc.tensor.matmul(warm_ps[:], ident[:], ident[:],
                                 start=True, stop=True)
            warm_sink = consts.tile([1, 1], f32, tag="warm_sink")
            nc.vector.tensor_copy(warm_sink[:], warm_ps[0:1, 0:1])

            # ---------------- weights ----------------
            winT = []
            weff = []
            gsl = []
            bsl = []
            for di in range(ND):
                wt = wp.tile([P, D], f32, tag=f"winT{di}")
                nc.scalar.dma_start(wt[:], winT_ext[di * P:(di + 1) * P, :])
                winT.append(wt)
                g = wp.tile([P, 1], f32, tag=f"gsl{di}")
                nc.scalar.dma_start(g[:], lng_ext[di * P:(di + 1) * P].unsqueeze(1))
                gsl.append(g)
                b = wp.tile([P, 1], f32, tag=f"bsl{di}")
                nc.scalar.dma_start(b[:], lnb_ext[di * P:(di + 1) * P].unsqueeze(1))
                bsl.append(b)
                we = wp.tile([P, D], CDT, tag=f"weff{di}")
                nc.vector.tensor_scalar_mul(we[:], wt[:], g[:])  # W_in^T * g (per-d row)
                weff.append(we)

            woutT = []
            bout = []
            for jt in range(ND):
                wf = work.tile([P, D], f32, tag="woutT_stage")
                nc.gpsimd.dma_start(wf[:], woutT_ext[jt * P:(jt + 1) * P, :])
                wo = wp.tile([P, D], CDT, tag=f"woutT{jt}")
                nc.vector.tensor_copy(wo[:], wf[:])
                woutT.append(wo)
                bo = wp.tile([P, 1], f32, tag=f"bout{jt}")
                nc.gpsimd.dma_start(bo[:], bout_ext[jt * P:(jt + 1) * P].unsqueeze(1))
                bout.append(bo)

            # bias_row[1, j] = ln_b @ W_in^T  (rank-1 LN-bias term of the projection)
            bias_ps = pss.tile([1, D], f32, tag="colA")
            for di in range(ND):
                nc.tensor.matmul(
                    bias_ps[:],
                    bsl[di][:],
                    winT[di][:],
                    start=(di == 0), stop=(di == ND - 1))
            bias_row = wp.tile([1, D], CDT, tag="bias_row")
            nc.scalar.copy(bias_row[:], bias_ps[:])

            # weight-predictor weights
            w1T = wp.tile([2 * DH, DH], f32, tag="w1T")
            nc.gpsimd.dma_start(w1T[:], w1T_ext[:])
            w1T_bf = wp.tile([2 * DH, DH], bf16, tag="w1T_bf")
            nc.vector.tensor_copy(w1T_bf[:], w1T[:])
            bm0 = wp.tile([P, 1], f32, tag="bm0")
            nc.gpsimd.dma_start(bm0[:], bmask_ext[0:1].to_broadcast((P, 1)))
            bm1 = wp.tile([P, 1], f32, tag="bm1")
            nc.gpsimd.dma_start(bm1[:], bmask_ext[1:2].to_broadcast((P, 1)))
            b1_rep = wp.tile([H, DH], f32, tag="b1_rep")
            nc.gpsimd.dma_start(b1_rep[:], b1_ext[None, :].to_broadcast((H, DH)))
            wlg_rep = wp.tile([H, DH], f32, tag="wlg_rep")
            nc.gpsimd.dma_start(wlg_rep[:], wlg_ext[None, :].to_broadcast((H, DH)))
            wlb_rep = wp.tile([H, DH], f32, tag="wlb_rep")
            nc.gpsimd.dma_start(wlb_rep[:], wlb_ext[None, :].to_broadcast((H, DH)))
            w2_rep = wp.tile([H, DH], f32, tag="w2_rep")
            nc.gpsimd.dma_start(w2_rep[:], w2_ext[None, :].to_broadcast((H, DH)))
            b2_col = wp.tile([H, 1], f32, tag="b2_col")
            nc.gpsimd.dma_start(b2_col[:], b2_ext[None, :].to_broadcast((H, 1)))

            # ---------------- per-tensor processing ----------------
            _ldq = [nc.sync, nc.scalar]
            _ldi = [0]

            def layernorm_transpose(x_ext, name, warm_mms=0):
                """LN one tensor; returns xnT [P(d), (di, rt, 128)]."""
                # xnT layout: [P(d-within-tile), (di, rt, 128)] columns
                xnT = keep.tile([P, ND * R], CDT, tag=f"xnT_{name}")
                xnT4 = xnT[:].rearrange("p (di r) -> p di r", di=ND)
                for rt in range(NT):
                    for wi in range(warm_mms):
                        wps = psum.tile([P, P], f32, tag="trps", bufs=3)
                        nc.tensor.matmul(wps[:], ident[:], ident[:],
                                         start=True, stop=True)
                    x_sb = work.tile([P, D], f32, tag="x_sb", bufs=6)
                    _ldq[_ldi[0] % 2].dma_start(x_sb[:], x_ext[rt * P:(rt + 1) * P, :])
                    _ldi[0] += 1
                    # LN stats (bn_stats -> (mean, var))
                    st6 = work.tile([P, 6], f32, tag="st6")
                    nc.vector.bn_stats(st6[:], x_sb[:])
                    mv = work.tile([P, 2], f32, tag="mv")
                    nc.vector.bn_aggr(mv[:], st6[:])
                    rstd = work.tile([P, 1], f32, tag="rstd")
                    nc.scalar.activation(rstd[:], mv[:, 1:2], AF.Sqrt, bias=eps_t[:])
                    nc.vector.reciprocal(rstd[:], rstd[:])
                    negm = work.tile([P, 1], f32, tag="negm")
                    nc.vector.tensor_scalar_mul(negm[:], mv[:, 0:1], -1.0)
                    xn = work.tile([P, D], CDT, tag="xn")
                    if rt % 2 == 0:
                        nmr = work.tile([P, 1], f32, tag="nmr")
                        nc.vector.tensor_tensor(nmr[:], negm[:], rstd[:], op=OP.mult)
                        nc.scalar.activation(xn[:], x_sb[:], AF.Identity,
                                             bias=nmr[:], scale=rstd[:])
                    else:
                        nc.gpsimd.tensor_scalar(
                            xn[:], x_sb[:], scalar1=negm[:], scalar2=rstd[:],
                            op0=OP.add, op1=OP.mult)
                    # transpose xn -> 4 di-blocks in one PSUM bank, one strided copy out
                    tr_ps = psum.tile([P, D], CDT, tag="trps", bufs=3)
                    for di in range(ND):
                        nc.tensor.transpose(
                            tr_ps[:, di * P:(di + 1) * P],
                            xn[:, di * P:(di + 1) * P], ident[:])
                    dst = xnT4[:, :, rt * P:(rt + 1) * P]
                    nc.scalar.copy(dst, tr_ps[:].rearrange("p (di r) -> p di r", di=ND))
                return xnT4

            def project_fx(xnT4, name, act_copy):
                """Project LN'd tensor: fx[rt] = xn @ (W_in*g)^T + ln_b @ W_in^T."""
                fx_tiles = []
                for rt in range(NT):
                    pj = psum.tile([P, D], f32, tag="projps", bufs=2)
                    for di in range(ND):
                        nc.tensor.matmul(
                            pj[:], xnT4[:, di, rt * P:(rt + 1) * P], weff[di][:],
                            start=(di == 0), stop=False)
                    nc.tensor.matmul(pj[:], ones_row[:], bias_row[:],
                                     start=False, stop=True)
                    fx = keep.tile([P, D], CDT, tag=f"fx_{name}{rt}")
                    if act_copy:
                        nc.scalar.copy(fx[:], pj[:])
                    else:
                        nc.vector.tensor_copy(fx[:], pj[:])
                    fx_tiles.append(fx)
                return fx_tiles

            def layernorm_project(x_ext, name, act_copy, warm_mms=0):
                return project_fx(layernorm_transpose(x_ext, name, warm_mms),
                                  name, act_copy)

            def rowstats_and_A(fx_tiles, name, gp_cov=False):
                """Per-row per-head inv-norm + neg-mean; build A [P, (h, 2*DH)]."""
                A_tiles = []
                cov_eng = nc.gpsimd if gp_cov else nc.vector
                for rt in range(NT):
                    fx = fx_tiles[rt]
                    fx3 = fx[:].rearrange("p (h c) -> p h c", h=H)
                    sqh = work.tile([P, D], CDT, tag="sqh")
                    nc.scalar.activation(sqh[:], fx[:], AF.Square)
                    qn2 = work.tile([P, H], f32, tag="qn2")
                    nc.vector.reduce_sum(
                        qn2[:], sqh[:].rearrange("p (h c) -> p h c", h=H), axis=AX.X)
                    qsum = work.tile([P, H], f32, tag="qsum")
                    nc.vector.reduce_sum(qsum[:], fx3, axis=AX.X)
                    invn = work.tile([P, H], f32, tag="invn")
                    nc.scalar.activation(invn[:], qn2[:], AF.Sqrt)
                    nc.vector.reciprocal(invn[:], invn[:])
                    hmean = work.tile([P, H], f32, tag="hmean")
                    nc.vector.tensor_scalar_mul(hmean[:], qsum[:], 1.0 / DH)
                    A = keep.tile([P, 2 * D], CDT, tag=f"A_{name}{rt}")
                    A4 = A[:].rearrange("p (h c) -> p h c", h=H)
                    nc.vector.tensor_tensor(
                        A4[:, :, 0:DH], fx3,
                        invn[:, :, None].broadcast_to((P, H, DH)), op=OP.mult)
                    cov_eng.tensor_tensor(
                        A4[:, :, DH:2 * DH], fx3,
                        hmean[:, :, None].broadcast_to((P, H, DH)), op=OP.subtract)
                    A_tiles.append(A)
                return A_tiles

            def colsums(fx_tiles, ps_tag):
                cs = pss.tile([1, D], f32, tag=ps_tag)
                for rt in range(NT):
                    nc.tensor.matmul(cs[:], colsum_lhsT[:], fx_tiles[rt][:],
                                     start=(rt == 0), stop=(rt == NT - 1))
                return cs

            # ---- k, v, q-LN back-to-back (keeps DVE feeding the PE);
            # k stats/A_k fill DVE afterwards while PE warms for S ----
            fk_tiles = layernorm_project(k_ext, "k", act_copy=True, warm_mms=4)
            fv_tiles = layernorm_project(v_ext, "v", act_copy=True)
            xnT4_q = layernorm_transpose(q_ext, "q")
            # qg_partial = (sum_rows xn_q / (B*N)) @ W_eff^T + (R/(B*N)) * bias_row
            xbar = keep.tile([P, ND], f32, tag="xbar")
            for di in range(ND):
                nc.vector.reduce_sum(xbar[:, di:di + 1], xnT4_q[:, di, :], axis=AX.X)
            xbar_bf = keep.tile([P, ND], bf16, tag="xbar_bf")
            nc.vector.tensor_scalar_mul(xbar_bf[:], xbar[:], 1.0 / (B * N))
            qg_ps = pss.tile([1, D], f32, tag="colC")
            for di in range(ND):
                nc.tensor.matmul(qg_ps[:], xbar_bf[:, di:di + 1], weff[di][:],
                                 start=(di == 0), stop=False)
            nc.tensor.matmul(qg_ps[:], eighth1[:], bias_row[:],
                             start=False, stop=True)
            kg_ps = colsums(fk_tiles, "colB")
            Ak = rowstats_and_A(fk_tiles, "k", gp_cov=True)
            # keep the PE clock warm while DVE builds A_k
            warm3_ps = psum.tile([P, P], f32, tag="trps", bufs=3)
            for wi in range(20):
                nc.tensor.matmul(warm3_ps[:], ident[:], fv_tiles[3][:, 0:P],
                                 start=True, stop=True)
            nc.vector.tensor_copy(warm_sink[:], warm3_ps[0:1, 0:1])
            s_ps = pss.tile([P, H * DH], f32, tag="colA")
            for h in range(H):
                for rt in range(NT):
                    nc.tensor.matmul(
                        s_ps[:, h * DH:(h + 1) * DH],
                        Ak[rt][:, h * 2 * DH:(h + 1) * 2 * DH],
                        fv_tiles[rt][:, h * DH:(h + 1) * DH],
                        start=(rt == 0), stop=(rt == NT - 1))
            # mask S partial into per-batch slots (one-hot bmask input per core)
            s_m0 = keep.tile([P, H * DH], bf16, tag="s_m0")
            nc.vector.tensor_scalar_mul(s_m0[:], s_ps[:], bm0[:])
            s_m1 = keep.tile([P, H * DH], bf16, tag="s_m1")
            nc.scalar.activation(s_m1[:], s_ps[:], AF.Copy, scale=bm1[:])
            ar1_sb = keep.tile([1, 2 * D], bf16, tag="ar1_sb")
            nc.scalar.copy(ar1_sb[:, 0:D], qg_ps[:])
            nc.scalar.copy(ar1_sb[:, D:2 * D], kg_ps[:])
            SEL = P * H * DH  # elements per S slot
            AR_E = 2 * SEL + 2 * D
            ar_in = dram.tile([AR_E], bf16, tag="ar_in")
            ar_out = dram.tile([AR_E], bf16, tag="ar_out")
            nc.sync.dma_start(ar_in[0:SEL].rearrange("(p f) -> p f", p=P), s_m0[:])
            nc.scalar.dma_start(ar_in[SEL:2 * SEL].rearrange("(p f) -> p f", p=P), s_m1[:])
            nc.gpsimd.dma_start(ar_in[2 * SEL:AR_E].unsqueeze(0), ar1_sb[:])
            nc.gpsimd.collective_compute(
                "AllReduce", OP.add,
                replica_groups=[list(range(N_CORES))],
                ins=[ar_in.opt()], outs=[ar_out.opt()])

            # ---- q projection + A_q transposes (overlap the collective) ----
            fq_tiles = project_fx(xnT4_q, "q", act_copy=True)
            Aq = rowstats_and_A(fq_tiles, "q")
            AqT = []
            for h in range(H):
                aq_ps = psum.tile([P, R], CDT, tag="trps", bufs=3)
                for rt in range(NT):
                    nc.tensor.transpose(
                        aq_ps[:, rt * P:(rt + 1) * P],
                        Aq[rt][:, h * 2 * DH:(h + 1) * 2 * DH], ident[:])
                at = keep.tile([P, R], CDT, tag=f"AqT{h}")
                if h % 2 == 0:
                    nc.vector.tensor_copy(at[:], aq_ps[:])
                else:
                    nc.scalar.copy(at[:], aq_ps[:])
                AqT.append(at)

            # ---- weight-predictor MLP (after AR1) ----
            featT = keep.tile([2 * DH, H], bf16, tag="featT")
            nc.sync.dma_start(
                featT[0:DH, :], ar_out[2 * SEL:2 * SEL + D].rearrange("(h c) -> c h", h=H))
            nc.scalar.dma_start(
                featT[DH:2 * DH, :], ar_out[2 * SEL + D:AR_E].rearrange("(h c) -> c h", h=H))
            hid_ps = pss.tile([H, DH], f32, tag="colA")
            nc.tensor.matmul(hid_ps[:], featT[:], w1T_bf[:],
                             start=True, stop=True)
            hid = keep.tile([H, DH], f32, tag="hid")
            nc.vector.tensor_tensor(hid[:], hid_ps[:], b1_rep[:], op=OP.add)
            # LN over DH (bn_stats)
            hst6 = keep.tile([H, 6], f32, tag="hst6")
            nc.vector.bn_stats(hst6[:], hid[:])
            hmv = keep.tile([H, 2], f32, tag="hmv")
            nc.vector.bn_aggr(hmv[:], hst6[:])
            hrstd = keep.tile([H, 1], f32, tag="hrstd")
            nc.scalar.activation(hrstd[:], hmv[:, 1:2], AF.Sqrt, bias=eps_t[0:H, :])
            nc.vector.reciprocal(hrstd[:], hrstd[:])
            hnmr = keep.tile([H, 1], f32, tag="hnmr")
            nc.vector.scalar_tensor_tensor(
                hnmr[:], hmv[:, 0:1], -1.0, hrstd[:], op0=OP.mult, op1=OP.mult)
            hln = keep.tile([H, DH], f32, tag="hln")
            nc.scalar.activation(hln[:], hid[:], AF.Identity,
                                 bias=hnmr[:], scale=hrstd[:])
            nc.vector.tensor_tensor(hln[:], hln[:], wlg_rep[:], op=OP.mult)
            nc.vector.tensor_tensor(hln[:], hln[:], wlb_rep[:], op=OP.add)
            nc.scalar.activation(hln[:], hln[:], AF.Relu)
            lscr = keep.tile([H, DH], f32, tag="lscr")
            nc.vector.tensor_tensor(lscr[:], hln[:], w2_rep[:], op=OP.mult)
            logit = keep.tile([H, 1], f32, tag="logit")
            nc.vector.reduce_sum(logit[:], lscr[:], axis=AX.X)
            nc.vector.tensor_tensor(logit[:], logit[:], b2_col[:], op=OP.add)
            wcol = keep.tile([H, 1], f32, tag="wcol")
            nc.scalar.activation(wcol[:], logit[:], AF.Sigmoid)
            wr_ps = pss.tile([1, H], f32, tag="colB")
            nc.tensor.transpose(wr_ps[:], wcol[:], ident8[:])
            wrow = keep.tile([1, H], f32, tag="wrow")
            nc.vector.tensor_copy(wrow[:], wr_ps[:])
            omw = keep.tile([1, H], bf16, tag="omw")
            nc.vector.tensor_scalar(omw[:], wrow[:], scalar1=-1.0, scalar2=1.0,
                                    op0=OP.mult, op1=OP.add)
            wdh = keep.tile([1, H], bf16, tag="wdh")
            nc.vector.tensor_scalar_mul(wdh[:], wrow[:], 1.0 / DH)
            wsc_ps = pss.tile([P, H], f32, tag="colB")
            nc.tensor.matmul(wsc_ps[:], ind_top[:], omw[:], start=True, stop=False)
            nc.tensor.matmul(wsc_ps[:], ind_bot[:], wdh[:], start=False, stop=True)
            wsc = keep.tile([P, H], bf16, tag="wsc")
            nc.vector.tensor_copy(wsc[:], wsc_ps[:])

            # ---- S readback, blend-scale, stage-8, final projection ----
            s0 = keep.tile([P, H * DH], bf16, tag="s0")
            nc.sync.dma_start(s0[:], ar_out[0:SEL].rearrange("(p f) -> p f", p=P))
            s1 = keep.tile([P, H * DH], bf16, tag="s1")
            nc.scalar.dma_start(s1[:], ar_out[SEL:2 * SEL].rearrange("(p f) -> p f", p=P))
            warm2_ps = pss.tile([P, P], f32, tag="colB")
            for wi in range(40):
                nc.tensor.matmul(warm2_ps[:], ident[:], s0[:, 0:P],
                                 start=True, stop=True)
            nc.vector.tensor_copy(warm_sink[:], warm2_ps[0:1, 0:1])
            sg0 = keep.tile([P, H * DH], bf16, tag="sg0")
            nc.vector.tensor_scalar_mul(sg0[:], s0[:], bm0[:])
            s_sum = keep.tile([P, H * DH], bf16, tag="s_sum")
            nc.vector.scalar_tensor_tensor(
                s_sum[:], s1[:], bm1[:], sg0[:], op0=OP.mult, op1=OP.add)
            s_sc = keep.tile([P, H * DH], CDT, tag="s_sc")
            nc.vector.tensor_tensor(
                s_sc[:].rearrange("p (h c) -> p h c", h=H),
                s_sum[:].rearrange("p (h c) -> p h c", h=H),
                wsc[:, :, None].broadcast_to((P, H, DH)), op=OP.mult)

            foutT = []
            for jt in range(ND):
                ft = keep.tile([P, R], CDT, tag=f"foutT{jt}")
                foutT.append(ft)
            for h in range(H):
                m_ps = psum.tile([DH, R], f32, tag="trps", bufs=3)
                nc.tensor.matmul(m_ps[:], s_sc[:, h * DH:(h + 1) * DH], AqT[h][:],
                                 start=True, stop=True)
                dst = foutT[h // 2][(h % 2) * DH:(h % 2) * DH + DH, :]
                if h % 2 == 0:
                    nc.scalar.copy(dst, m_ps[:])
                else:
                    nc.vector.tensor_copy(dst, m_ps[:])

            for dt_ in range(ND):
                o_ps = psum.tile([P, R], f32, tag="projps", bufs=2)
                for jt in range(ND):
                    nc.tensor.matmul(
                        o_ps[:], woutT[jt][:, dt_ * P:(dt_ + 1) * P], foutT[jt][:],
                        start=(jt == 0), stop=(jt == ND - 1))
                o_sb = work.tile([P, R], f32, tag="o_sb")
                if dt_ % 2 == 0:
                    nc.scalar.activation(o_sb[:], o_ps[:], AF.Identity,
                                         bias=bout[dt_][:], scale=1.0)
                else:
                    nc.vector.tensor_scalar_add(o_sb[:], o_ps[:], bout[dt_][:])
                _ldq[dt_ % 2].dma_start(out_ext[dt_ * P:(dt_ + 1) * P, :], o_sb[:])

    nc.finalize()
    return nc


def _get_program():
    if "nc" not in _CACHE:
        _CACHE["nc"] = _build_program()
    return _CACHE["nc"]


def _make_in_maps(inputs):
    q = np.ascontiguousarray(np.asarray(inputs["q"], np.float32).reshape(B * N, D))
    k = np.ascontiguousarray(np.asarray(inputs["k"], np.float32).reshape(B * N, D))
    v = np.ascontiguousarray(np.asarray(inputs["v"], np.float32).reshape(B * N, D))
    shared = {
        "W_inT": np.ascontiguousarray(np.asarray(inputs["W_in"], np.float32).T),
        "W_outT": np.ascontiguousarray(np.asarray(inputs["W_out"], np.float32).T),
        "ln_g": np.asarray(inputs["ln_g"], np.float32),
        "ln_b": np.asarray(inputs["ln_b"], np.float32),
        "b_out": np.asarray(inputs["b_out"], np.float32),
        "wp_w1T": np.ascontiguousarray(np.asarray(inputs["wp_w1"], np.float32).T),
        "wp_b1": np.asarray(inputs["wp_b1"], np.float32),
        "wp_ln_g": np.asarray(inputs["wp_ln_g"], np.float32),
        "wp_ln_b": np.asarray(inputs["wp_ln_b"], np.float32),
        "wp_w2": np.ascontiguousarray(np.asarray(inputs["wp_w2"], np.float32).reshape(DH)),
        "wp_b2": np.asarray(inputs["wp_b2"], np.float32).reshape(1),
    }
    in_maps = []
    for c in range(N_CORES):
        m = dict(shared)
        sl = slice(c * R, (c + 1) * R)
        m["q"] = np.ascontiguousarray(q[sl])
        m["k"] = np.ascontiguousarray(k[sl])
        m["v"] = np.ascontiguousarray(v[sl])
        b = (c * R) // N
        m["bmask"] = np.eye(2, dtype=np.float32)[b]
        in_maps.append(m)
    return in_maps


def _gather(results):
    out = np.empty((B * N, D), np.float32)
    for c in range(N_CORES):
        out[c * R:(c + 1) * R, :] = results[c]["out"].T
    return out.reshape(B, N, D)


def _run(inputs, trace=False, trace_cores=None):
    from concourse.bass_utils import run_bass_kernel_spmd
    nc = _get_program()
    in_maps = _make_in_maps(inputs)
    res = run_bass_kernel_spmd(
        nc, in_maps, core_ids=list(range(N_CORES)),
        trace=trace, trace_cores=trace_cores)
    return _gather(res.results), res


def kernel(**inputs) -> np.ndarray:
    out, _ = _run(inputs, trace=False)
    return out


def run_traced(inputs, trace_cores=None):
    return _run(inputs, trace=True, trace_cores=trace_cores)

